# revision 20
# baseline (speedup 1.0000x reference)
"""Trainium2 Bass kernel for nn_Attention (B=8, N=1024, C=768, H=12).

Data-parallel over batch: core b handles batch element b.

Math (re-associated to avoid the huge bhqk,bhqd->bkd contraction):
  q = x Wq^T, k = x Wk^T             (per head h: qh, kh  [N, Z])
  S_h = qh kh^T * scale              [N, N]
  E_h = exp(S_h), den = rowsum(E_h)
  AT_h = [E^T (k/den) ; E^T (q/den)]^T   [2Z, N]
  out  = sum_h AT_h^T @ M_hT + bp    with M_h = [Wq_h;Wk_h] @ Wp^T

Structure (round 2'):
  - 6 head-PAIR phases; the two heads' score matmuls (K=Z=64) live in
    disjoint PE row groups (rows 0-63 / 64-127 via base_partition) and
    interleave -> concurrent in the PE array (~2x score throughput).
  - AT accumulation in fp8e4m3 DoubleRow (contraction 256 = 2 q-tiles
    per MM): E written as fp8 by the ACT exp, kqs = knat*rv*2^15 in fp8
    (2^15 compensated by a 2^-15 scale on the at_ps -> AT_sb copy).
    Projections and phase C stay bf16: their errors feed the output
    linearly (no softmax averaging) and fp8 there blows the error
    budget (verified against a numpy model of the full pipeline).
  - exp split: ACT handles 12/16 tiles per pair (fused exp+den via
    accum_out); the DVE handles u-groups (par0,u1) t=2,3 and (par1,u2)
    t=4,5 via a bf16 Schraudolph bit-trick + row-sum, consumed by bf16
    (non-DR) AT matmuls.  The split staggers ACT/DVE within a t-step.
  - AT work for pair j drains granularly (one u-group closure per
    t-step) through pair j+1; the last pair uses per-u reciprocals and
    drains its own AT work immediately to shorten the tail.
"""

import sys
from contextlib import ExitStack

import numpy as np

if "/opt/trn_rl_repo" not in sys.path:
    sys.path.insert(0, "/opt/trn_rl_repo")

import ml_dtypes
import concourse.bass as bass
import concourse.mybir as mybir
import concourse.tile as tile
from concourse import bacc, bass_utils
from concourse.bass import ts

B, N, C, H = 8, 1024, 768, 12
Z = C // H          # 64
P = 128
NT = N // P         # 8 qi tiles
CT = C // P         # 6 c tiles
NP = H // 2         # 6 head pairs
NU = NT // 2        # 4 q-tile pairs (DoubleRow u-groups)
SCALE = Z ** -0.5   # 0.125
FP = mybir.dt.float32
BF = mybir.dt.bfloat16
F8 = mybir.dt.float8e4
I16 = mybir.dt.int16
DR = mybir.MatmulPerfMode.DoubleRow

KQS_SH = 15         # kqs = knat * rv * 2^15 (fp8 range); at_copy * 2^-15
CCH = [(0, 512), (512, 256)]  # C=768 split into matmul free-dim chunks

# bf16 Schraudolph bit-trick exp for the DVE tiles
EXP_K1 = SCALE * np.log2(np.e) * 128.0
EXP_K2 = 16256.0 - 0.0436 * 128.0

# (parity, u) q-tile pairs handled by the DVE (bf16 E, non-DR AT).
# Pairs 0-4 put the second DVE group at the pair's end so the ACT load
# per step runs 2,2,1,1,2,2,1,1 -- a light seam overlaps the next
# pair's first scores.  The last pair keeps u=2 so its final q-tiles
# stay on the fused-accum ACT path (shorter tail).
DVE_U = {(0, 1), (1, 3)}
DVE_U_LAST = {(0, 1), (1, 2)}

last_results = None  # set by kernel() for test harness introspection


def emit(ctx: ExitStack, tc: tile.TileContext, io):
    nc = tc.nc
    xT, wqkT, M, bpr, out = io

    stack = []  # (name, free) in creation order; freed strictly LIFO

    def single(shape, dtype, name):
        t, free = tc.tile(shape, dtype, name=name)
        stack.append((name, free))
        return t

    def free_through(name):
        while stack:
            nm, fr = stack.pop()
            fr()
            if nm == name:
                return
        raise KeyError(name)

    # ------- PSUM pools: scores 3x2 banks + at 1 + chains 1 = 8 --------
    # scores keep all 3 double-bank ring buffers to themselves; the
    # projection chains and the chunk-split AT accumulator get their own
    # single-bank pools so they never throttle the exp pipeline's
    # lookahead.
    psS = ctx.enter_context(tc.tile_pool(name="psS", bufs=3, space="PSUM"))
    psA = ctx.enter_context(tc.tile_pool(name="psA", bufs=1, space="PSUM"))
    psC = ctx.enter_context(tc.tile_pool(name="psC", bufs=1, space="PSUM"))

    def ps_tile():
        return psS.tile([P, N], FP, name="s", tag="s")

    # SBUF pools (entered before any single so LIFO holds at ctx exit)
    p_E = ctx.enter_context(tc.tile_pool(name="p_E", bufs=12))
    p_Eb = ctx.enter_context(tc.tile_pool(name="p_Eb", bufs=4))
    p_kqs = ctx.enter_context(tc.tile_pool(name="p_kqs", bufs=12))
    p_den = ctx.enter_context(tc.tile_pool(name="p_den", bufs=8))
    p_out = ctx.enter_context(tc.tile_pool(name="p_out", bufs=4))

    # ------------- singles, bottom of stack = longest-lived -------------
    M_all = single([P, H * C], BF, name="M_all")
    M_sb = [M_all[:, ts(h, C)] for h in range(H)]
    bp_sb = single([1, C], BF, name="bp_sb")
    ones_sb = single([1, P], BF, name="ones_sb")
    nc.gpsimd.memset(ones_sb[:], 1.0)
    AT_sb = [single([P, N], BF, name=f"AT{h}") for h in range(H)]
    # natkq[j]: [128, 2N] cols 0:N = k natural (t-major 128-col blocks),
    # N:2N = q natural; features c of heads 2j, 2j+1.
    natkq = [single([P, 2 * N], BF, name=f"natkq{j}") for j in range(CT)]
    qT_sb = [single([P, N], BF, name=f"qT{j}") for j in range(CT)]
    kT_sb = [single([P, N], BF, name=f"kT{j}") for j in range(CT)]
    wqkT_all = single([P, CT * 2 * C], BF, name="wqkT_all")

    def w_sl(k, j, which):
        off = k * 2 * C + C * which + P * j
        return wqkT_all[:, off:off + P]

    xT_all = single([P, CT * N], BF, name="xT_all")
    xT_sb = [xT_all[:, ts(i, N)] for i in range(CT)]

    # DRAM scratch for the qT/kT -> natural-layout xbar transposes
    qkTd = []
    for j in range(CT):
        t_, _free = tc.tile([2, P, N], BF, space="DRAM", name=f"qkTd{j}")
        qkTd.append(t_)

    # HAM keep-warm scratch: dummy matmuls hold the PE at 2.4 GHz
    # through the input-DMA window.
    warm_sb = single([P, 512], BF, name="warm_sb")
    nc.gpsimd.memset(warm_sb[:], 0)

    def dummy_mms(n):
        ps = ps_tile()
        for i in range(n):
            nc.tensor.matmul(ps[:, 0:512], lhsT=warm_sb[:, 0:P],
                             rhs=warm_sb[:], start=(i == 0), stop=(i == n - 1))

    # ---------------- batched input DMAs (phase-A inputs first) ---------
    # two parallel HWDGE streams: x chunks on the SP queue, weights on
    # the Activation queue (idle until the first exp, ~15us later)
    for k in range(CT):
        nc.sync.dma_start(xT_sb[k][:], xT[ts(k, P), :])
        nc.scalar.dma_start(wqkT_all[:, ts(k, 2 * C)], wqkT[ts(k, P), :])
    nc.scalar.dma_start(M_all[:], M[:])
    nc.scalar.dma_start(bp_sb[:], bpr[:])

    # ---------------- projection chains ----------------
    def chain(dst_ap, lhsT_of, rhs_of, width):
        ps = psC.tile([P, 512], FP, name="c", tag="c")
        for k in range(CT):
            nc.tensor.matmul(
                ps[:, 0:width],
                lhsT=lhsT_of(k),
                rhs=rhs_of(k),
                start=(k == 0),
                stop=(k == CT - 1),
            )
        nc.vector.tensor_copy(dst_ap, ps[:, 0:width])

    def qkT_chains(j):
        def one(which, ch):
            cols = slice(512 * ch, 512 * ch + 512)
            dst = (qT_sb if which == 0 else kT_sb)[j][:, cols]
            chain(dst,
                  lambda k: w_sl(k, j, which),
                  lambda k: xT_sb[k][:, cols], 512)
        return [lambda w=w, c=c: one(w, c) for w, c in
                [(1, 0), (0, 0), (1, 1), (0, 1)]]

    def emit_nat_dma(j):
        """qT/kT[j] -> DRAM -> xbar-transposed natural layout natkq[j]."""
        nc.sync.dma_start(qkTd[j][1], kT_sb[j][:])
        nc.sync.dma_start(qkTd[j][0], qT_sb[j][:])
        nc.sync.dma_start_transpose(
            natkq[j][:, 0:N].rearrange("p (t c) -> p t c", c=P),
            qkTd[j][1].rearrange("c (t q) -> c t q", q=P))
        nc.sync.dma_start_transpose(
            natkq[j][:, N:2 * N].rearrange("p (t c) -> p t c", c=P),
            qkTd[j][0].rearrange("c (t q) -> c t q", q=P))

    # preload the ACT exp table set during the input-DMA window (the
    # first Exp otherwise pays the ~2.7us ACT_TABLE_LOAD on the
    # critical path)
    tbl_warm = p_den.tile([1, 8], BF, name="tblw")
    nc.scalar.activation(tbl_warm[:], warm_sb[0:1, 0:8],
                         mybir.ActivationFunctionType.Exp)
    # one short PE warm burst; the chains themselves keep HAM busy once
    # their input chunks land (24 cold dummies ahead of the chains cost
    # ~10us of lead-in on the serial PE queue)
    dummy_mms(8)
    for th in qkT_chains(0):
        th()
    emit_nat_dma(0)

    # ---------------- phase B: 6 pair-phases ----------------------------
    at_queue = []

    def drain_at(n):
        while len(at_queue) > n:
            at_queue.pop(0)()

    # chain fragments for pair j+1, spread across pair j's t-steps
    def chain_frags(j, which, ch):
        cols = slice(512 * ch, 512 * ch + 512)
        dst = (qT_sb if which == 0 else kT_sb)[j][:, cols]
        box = {}

        def f1():
            ps = psC.tile([P, 512], FP, name="c", tag="c")
            box["ps"] = ps
            for k in range(3):
                nc.tensor.matmul(
                    ps[:, 0:512],
                    lhsT=w_sl(k, j, which),
                    rhs=xT_sb[k][:, cols], start=(k == 0), stop=False)

        def f2():
            ps = box["ps"]
            for k in range(3, 6):
                nc.tensor.matmul(
                    ps[:, 0:512],
                    lhsT=w_sl(k, j, which),
                    rhs=xT_sb[k][:, cols], start=False, stop=(k == 5))
            nc.vector.tensor_copy(dst, ps[:, 0:512])

        return f1, f2

    for pj in range(NP):
        last_pair = pj == NP - 1
        dve_u = DVE_U_LAST if last_pair else DVE_U
        qt, kt = qT_sb[pj], kT_sb[pj]
        nat3 = natkq[pj].rearrange("p (g t c) -> p g t c", g=2, c=P)
        den = [p_den.tile([P, NT], FP, name=f"den{par}") for par in range(2)]
        rv = [p_den.tile([P, NT], FP, name=f"rv{par}") for par in range(2)]
        Ep = [[None] * NU for _ in range(2)]
        kqs_t = [[None] * NU for _ in range(2)]

        def kqs_u(par, u, nat3=nat3, rv=rv, kqs_t=kqs_t, dve_u=dve_u):
            # DVE-only: safe to run inline right after the per-u recip
            dve = (par, u) in dve_u
            kqs = p_kqs.tile([P, 2, 2 * Z], BF if dve else F8, name="kqst")
            kqs_t[par][u] = kqs
            for i in range(2):
                t = 2 * u + i
                nc.vector.tensor_scalar(
                    kqs[:, i, :].rearrange("p (g z) -> p g z", g=2),
                    nat3[:, :, t, ts(par, Z)],
                    rv[par][:, t:t + 1], float(1 << KQS_SH),
                    op0=mybir.AluOpType.mult,
                    op1=mybir.AluOpType.mult)

        def at_chunk(par, ch, pj=pj, Ep=Ep, kqs_t=kqs_t,
                     last_pair=last_pair, dve_u=dve_u):
            # one 512-col half of a head's AT accumulation: 4 DR matmuls
            # (8 bf16 for DVE u-groups) through a 1-bank buffer, then a
            # scaled PSUM->SBUF evacuation of that half.  The last
            # pair's par-1 half uses the chain bank (free by then) so
            # the two heads' tail chains run decoupled.
            def work():
                if last_pair and par == 1:
                    at_ps = psC.tile([P, 512], FP, name="c", tag="c")
                else:
                    at_ps = psA.tile([P, 512], FP, name="at", tag="at")
                cols = slice(512 * ch, 512 * ch + 512)
                for u in range(NU):
                    if (par, u) in dve_u:
                        for i in range(2):
                            nc.tensor.matmul(
                                at_ps[:], lhsT=kqs_t[par][u][:, i, :],
                                rhs=Ep[par][u][:, i, cols],
                                start=(u == 0 and i == 0),
                                stop=(u == NU - 1 and i == 1),
                            )
                    else:
                        nc.tensor.matmul(
                            at_ps[:], lhsT=kqs_t[par][u][:, 0:2, :],
                            rhs=Ep[par][u][:, 0:2, cols],
                            start=(u == 0), stop=(u == NU - 1),
                            perf_mode=DR,
                        )
                h = 2 * pj + par
                if ch == 0:
                    nc.scalar.mul(AT_sb[h][:, cols], at_ps[:],
                                  2.0 ** (-KQS_SH))
                else:
                    nc.vector.tensor_scalar_mul(AT_sb[h][:, cols], at_ps[:],
                                                2.0 ** (-KQS_SH))
            return work

        # extras: proj chains + nat DMA for pair pj+1
        ext = []
        if not last_pair:
            for which, ch in [(1, 0), (0, 0), (1, 1), (0, 1)]:
                ext.extend(chain_frags(pj + 1, which, ch))
            ext.append(lambda j=pj: emit_nat_dma(j + 1))

        for t in range(NT):
            u, i = t // 2, t & 1
            S2 = []
            for par in range(2):
                S = ps_tile()
                S2.append(S)
                if i == 0:
                    dve = (par, u) in dve_u
                    Ep[par][u] = (p_Eb.tile([P, 2, N], BF, name="Eb")
                                  if dve else p_E.tile([P, 2, N], F8, name="Ep"))
            # interleaved score MMs: the two heads target PE row groups
            # (0,0) / (64,0) (auto-derived from base_partition) and run
            # concurrently in the array
            for ch in range(2):
                cols = slice(512 * ch, 512 * ch + 512)
                for par in range(2):
                    base = Z * par
                    nc.tensor.matmul(
                        S2[par][:, cols],
                        lhsT=qt[base:base + Z, ts(t, P)],
                        rhs=kt[base:base + Z, cols],
                        start=True, stop=True,
                    )
            for par in range(2):
                E_ap = Ep[par][u][:, i, :]
                if (par, u) in dve_u:
                    # bf16 Schraudolph bit-trick + DVE row-sum
                    nc.vector.tensor_scalar(
                        E_ap.bitcast(I16), S2[par][:], EXP_K1, EXP_K2,
                        op0=mybir.AluOpType.mult, op1=mybir.AluOpType.add)
                    nc.vector.tensor_reduce(
                        den[par][:, t:t + 1], E_ap,
                        axis=mybir.AxisListType.X, op=mybir.AluOpType.add)
                else:
                    nc.scalar.activation(
                        E_ap, S2[par][:], mybir.ActivationFunctionType.Exp,
                        scale=SCALE, accum_out=den[par][:, t:t + 1],
                    )
            # per-u reciprocals + inline kqs (DVE-only, ready right
            # after the u-group's second exp)
            if i == 1:
                for par in range(2):
                    nc.vector.reciprocal(rv[par][:, 2 * u:2 * u + 2],
                                         den[par][:, 2 * u:2 * u + 2])
                    kqs_u(par, u)
            if last_pair:
                drain_at(1 if t < NT - 1 else 0)
            else:
                drain_at(3 - t // 2)
            for _ in range(2):
                if ext:
                    ext.pop(0)()

        # this pair's AT matmul chunks drain through the next pair
        for par in range(2):
            for ch in range(2):
                at_queue.append(at_chunk(par, ch))
    drain_at(0)

    free_through("natkq0")  # frees xT, wqkT, kT*, qT*, natkq*, warm_sb

    # ---------------- phase C: fused combine + projection + bias ------
    # bias folded in as a K=1 matmul (ones^T @ bp_row) so the PSUM
    # evacuation is a plain copy, alternated between ACT and DVE
    for t in range(NT):
        F_ps = ps_tile()
        for h in range(H):
            for off, w in CCH:
                nc.tensor.matmul(
                    F_ps[:, off:off + w],
                    lhsT=AT_sb[h][:, ts(t, P)],
                    rhs=M_sb[h][:, off:off + w],
                    start=(h == 0), stop=False,
                )
        for off, w in CCH:
            nc.tensor.matmul(
                F_ps[:, off:off + w],
                lhsT=ones_sb[:],
                rhs=bp_sb[0:1, off:off + w],
                start=False, stop=True,
            )
        o = p_out.tile([P, C], FP, name="outt")
        if t % 2 == 0:
            nc.scalar.copy(o[:], F_ps[:, 0:C])
        else:
            nc.vector.tensor_copy(o[:], F_ps[:, 0:C])
        nc.sync.dma_start(out[ts(t, P), :], o[:])

    while stack:
        stack.pop()[1]()


def build():
    nc = bacc.Bacc("TRN2", target_bir_lowering=False, debug=False, num_devices=B)
    xT = nc.dram_tensor("xT", [C, N], BF, kind="ExternalInput").ap()
    wqkT = nc.dram_tensor("wqkT", [C, 2 * C], BF, kind="ExternalInput").ap()
    M = nc.dram_tensor("M", [P, H * C], BF, kind="ExternalInput").ap()
    bpr = nc.dram_tensor("bpr", [1, C], BF, kind="ExternalInput").ap()
    out = nc.dram_tensor("out", [N, C], FP, kind="ExternalOutput").ap()
    with tile.TileContext(nc) as tc, ExitStack() as ctx:
        emit(ctx, tc, (xT, wqkT, M, bpr, out))
    nc.compile()
    return nc


def kernel(x, Wq, Wk, Wp, bp, trace=False, **trace_kwargs):
    global last_results
    x = np.asarray(x, dtype=np.float32)
    Wq = np.asarray(Wq, dtype=np.float32)
    Wk = np.asarray(Wk, dtype=np.float32)
    Wp = np.asarray(Wp, dtype=np.float32)
    bp = np.asarray(bp, dtype=np.float32)

    nc = build()
    bf = ml_dtypes.bfloat16
    wqkTc = np.ascontiguousarray(
        np.concatenate([Wq.T, Wk.T], axis=1)).astype(bf)  # [C, 2C]
    # fused combine+projection weights: M_hT = [Wq_h; Wk_h] @ Wp^T  [2Z, C]
    Wq_h = Wq.reshape(H, Z, C)
    Wk_h = Wk.reshape(H, Z, C)
    W2 = np.concatenate([Wq_h, Wk_h], axis=1)             # [H, 2Z, C]
    M_np = np.einsum("hzc,dc->hzd", W2, Wp)               # [H, 2Z, C]
    Mc = np.ascontiguousarray(
        M_np.transpose(1, 0, 2).reshape(P, H * C)).astype(bf)
    bprc = np.ascontiguousarray(bp.reshape(1, C).astype(bf))
    in_maps = []
    for b in range(B):
        in_maps.append({
            "xT": np.ascontiguousarray(x[b].T).astype(bf),
            "wqkT": wqkTc, "M": Mc, "bpr": bprc,
        })
    res = bass_utils.run_bass_kernel_spmd(
        nc, in_maps, core_ids=list(range(B)), trace=trace, **trace_kwargs)
    last_results = res
    return np.stack([res.results[b]["out"] for b in range(B)], axis=0)


# revision 21
# speedup vs baseline: 1.1744x; 1.1744x over previous
"""Trainium2 Bass kernel for nn_Attention (B=8, N=1024, C=768, H=12).

Data-parallel over batch: core b handles batch element b.

Math (re-associated to avoid the huge bhqk,bhqd->bkd contraction):
  q = x Wq^T, k = x Wk^T             (per head h: qh, kh  [N, Z])
  S_h = qh kh^T * scale              [N, N]
  E_h = exp(S_h), den = rowsum(E_h)
  AT_h = [E^T (k/den) ; E^T (q/den)]^T   [2Z, N]
  out  = sum_h AT_h^T @ M_hT + bp    with M_h = [Wq_h;Wk_h] @ Wp^T

Structure (round 2'):
  - 6 head-PAIR phases; the two heads' score matmuls (K=Z=64) live in
    disjoint PE row groups (rows 0-63 / 64-127 via base_partition) and
    interleave -> concurrent in the PE array (~2x score throughput).
  - AT accumulation in fp8e4m3 DoubleRow (contraction 256 = 2 q-tiles
    per MM): E written as fp8 by the ACT exp, kqs = knat*rv*2^15 in fp8
    (2^15 compensated by a 2^-15 scale on the at_ps -> AT_sb copy).
    Projections and phase C stay bf16: their errors feed the output
    linearly (no softmax averaging) and fp8 there blows the error
    budget (verified against a numpy model of the full pipeline).
  - exp split: ACT handles 12/16 tiles per pair (fused exp+den via
    accum_out); the DVE handles u-groups (par0,u1) t=2,3 and (par1,u2)
    t=4,5 via a bf16 Schraudolph bit-trick + row-sum, consumed by bf16
    (non-DR) AT matmuls.  The split staggers ACT/DVE within a t-step.
  - AT work for pair j drains granularly (one u-group closure per
    t-step) through pair j+1; the last pair uses per-u reciprocals and
    drains its own AT work immediately to shorten the tail.
"""

import sys
from contextlib import ExitStack

import numpy as np

if "/opt/trn_rl_repo" not in sys.path:
    sys.path.insert(0, "/opt/trn_rl_repo")

import ml_dtypes
import concourse.bass as bass
import concourse.mybir as mybir
import concourse.tile as tile
from concourse import bacc, bass_utils
from concourse.bass import ts

B, N, C, H = 8, 1024, 768, 12
Z = C // H          # 64
P = 128
NT = N // P         # 8 qi tiles
CT = C // P         # 6 c tiles
NP = H // 2         # 6 head pairs
NU = NT // 2        # 4 q-tile pairs (DoubleRow u-groups)
SCALE = Z ** -0.5   # 0.125
FP = mybir.dt.float32
BF = mybir.dt.bfloat16
F8 = mybir.dt.float8e4
I16 = mybir.dt.int16
DR = mybir.MatmulPerfMode.DoubleRow

KQS_SH = 15         # kqs = knat * rv * 2^15 (fp8 range); at_copy * 2^-15
CCH = [(0, 512), (512, 256)]  # C=768 split into matmul free-dim chunks

# bf16 Schraudolph bit-trick exp for the DVE tiles
EXP_K1 = SCALE * np.log2(np.e) * 128.0
EXP_K2 = 16256.0 - 0.0436 * 128.0

# (parity, u) q-tile pairs handled by the DVE (bf16 E, non-DR AT)
DVE_U = {(0, 1), (1, 2)}

last_results = None  # set by kernel() for test harness introspection


def emit(ctx: ExitStack, tc: tile.TileContext, io):
    nc = tc.nc
    xT, wqkT, M, bpr, out = io

    stack = []  # (name, free) in creation order; freed strictly LIFO

    def single(shape, dtype, name):
        t, free = tc.tile(shape, dtype, name=name)
        stack.append((name, free))
        return t

    def free_through(name):
        while stack:
            nm, fr = stack.pop()
            fr()
            if nm == name:
                return
        raise KeyError(name)

    # ------- PSUM pools: scores 3x2 banks + at 1 + chains 1 = 8 --------
    # scores keep all 3 double-bank ring buffers to themselves; the
    # projection chains and the chunk-split AT accumulator get their own
    # single-bank pools so they never throttle the exp pipeline's
    # lookahead.
    psS = ctx.enter_context(tc.tile_pool(name="psS", bufs=3, space="PSUM"))
    psA = ctx.enter_context(tc.tile_pool(name="psA", bufs=1, space="PSUM"))
    psC = ctx.enter_context(tc.tile_pool(name="psC", bufs=1, space="PSUM"))

    def ps_tile():
        return psS.tile([P, N], FP, name="s", tag="s")

    # SBUF pools (entered before any single so LIFO holds at ctx exit)
    p_E = ctx.enter_context(tc.tile_pool(name="p_E", bufs=12))
    p_Eb = ctx.enter_context(tc.tile_pool(name="p_Eb", bufs=4))
    p_kqs = ctx.enter_context(tc.tile_pool(name="p_kqs", bufs=12))
    p_den = ctx.enter_context(tc.tile_pool(name="p_den", bufs=8))
    p_out = ctx.enter_context(tc.tile_pool(name="p_out", bufs=4))

    # ------------- singles, bottom of stack = longest-lived -------------
    M_all = single([P, H * C], BF, name="M_all")
    M_sb = [M_all[:, ts(h, C)] for h in range(H)]
    bp_sb = single([1, C], BF, name="bp_sb")
    ones_sb = single([1, P], BF, name="ones_sb")
    nc.gpsimd.memset(ones_sb[:], 1.0)
    AT_sb = [single([P, N], BF, name=f"AT{h}") for h in range(H)]
    # natkq[j]: [128, 2N] cols 0:N = k natural (t-major 128-col blocks),
    # N:2N = q natural; features c of heads 2j, 2j+1.
    natkq = [single([P, 2 * N], BF, name=f"natkq{j}") for j in range(CT)]
    qT_sb = [single([P, N], BF, name=f"qT{j}") for j in range(CT)]
    kT_sb = [single([P, N], BF, name=f"kT{j}") for j in range(CT)]
    wqkT_all = single([P, CT * 2 * C], BF, name="wqkT_all")

    def w_sl(k, j, which):
        off = k * 2 * C + C * which + P * j
        return wqkT_all[:, off:off + P]

    xT_all = single([P, CT * N], BF, name="xT_all")
    xT_sb = [xT_all[:, ts(i, N)] for i in range(CT)]

    # DRAM scratch for the qT/kT -> natural-layout xbar transposes
    qkTd = []
    for j in range(CT):
        t_, _free = tc.tile([2, P, N], BF, space="DRAM", name=f"qkTd{j}")
        qkTd.append(t_)

    # HAM keep-warm scratch: dummy matmuls hold the PE at 2.4 GHz
    # through the input-DMA window.
    warm_sb = single([P, 512], BF, name="warm_sb")
    nc.gpsimd.memset(warm_sb[:], 0)

    def dummy_mms(n):
        ps = ps_tile()
        for i in range(n):
            nc.tensor.matmul(ps[:, 0:512], lhsT=warm_sb[:, 0:P],
                             rhs=warm_sb[:], start=(i == 0), stop=(i == n - 1))

    # ---------------- batched input DMAs (phase-A inputs first) ---------
    # two parallel HWDGE streams: x chunks on the SP queue, weights on
    # the Activation queue (idle until the first exp, ~15us later)
    for k in range(CT):
        nc.sync.dma_start(xT_sb[k][:], xT[ts(k, P), :])
        nc.scalar.dma_start(wqkT_all[:, ts(k, 2 * C)], wqkT[ts(k, P), :])
    nc.scalar.dma_start(M_all[:], M[:])
    nc.scalar.dma_start(bp_sb[:], bpr[:])

    # ---------------- projection chains ----------------
    def chain(dst_ap, lhsT_of, rhs_of, width):
        ps = psC.tile([P, 512], FP, name="c", tag="c")
        for k in range(CT):
            nc.tensor.matmul(
                ps[:, 0:width],
                lhsT=lhsT_of(k),
                rhs=rhs_of(k),
                start=(k == 0),
                stop=(k == CT - 1),
            )
        nc.vector.tensor_copy(dst_ap, ps[:, 0:width])

    def qkT_chains(j):
        def one(which, ch):
            cols = slice(512 * ch, 512 * ch + 512)
            dst = (qT_sb if which == 0 else kT_sb)[j][:, cols]
            chain(dst,
                  lambda k: w_sl(k, j, which),
                  lambda k: xT_sb[k][:, cols], 512)
        return [lambda w=w, c=c: one(w, c) for w, c in
                [(1, 0), (0, 0), (1, 1), (0, 1)]]

    def emit_nat_dma(j):
        """qT/kT[j] -> DRAM -> xbar-transposed natural layout natkq[j]."""
        nc.sync.dma_start(qkTd[j][1], kT_sb[j][:])
        nc.sync.dma_start(qkTd[j][0], qT_sb[j][:])
        nc.sync.dma_start_transpose(
            natkq[j][:, 0:N].rearrange("p (t c) -> p t c", c=P),
            qkTd[j][1].rearrange("c (t q) -> c t q", q=P))
        nc.sync.dma_start_transpose(
            natkq[j][:, N:2 * N].rearrange("p (t c) -> p t c", c=P),
            qkTd[j][0].rearrange("c (t q) -> c t q", q=P))

    # preload the ACT exp table set during the input-DMA window (the
    # first Exp otherwise pays the ~2.7us ACT_TABLE_LOAD on the
    # critical path)
    tbl_warm = p_den.tile([1, 8], BF, name="tblw")
    nc.scalar.activation(tbl_warm[:], warm_sb[0:1, 0:8],
                         mybir.ActivationFunctionType.Exp)
    # one short PE warm burst; the chains themselves keep HAM busy once
    # their input chunks land (24 cold dummies ahead of the chains cost
    # ~10us of lead-in on the serial PE queue)
    dummy_mms(8)
    for th in qkT_chains(0):
        th()
    emit_nat_dma(0)

    # ---------------- phase B: 6 pair-phases ----------------------------
    at_queue = []

    def drain_at(n):
        while len(at_queue) > n:
            at_queue.pop(0)()

    # chain fragments for pair j+1, spread across pair j's t-steps
    def chain_frags(j, which, ch):
        cols = slice(512 * ch, 512 * ch + 512)
        dst = (qT_sb if which == 0 else kT_sb)[j][:, cols]
        box = {}

        def f1():
            ps = psC.tile([P, 512], FP, name="c", tag="c")
            box["ps"] = ps
            for k in range(3):
                nc.tensor.matmul(
                    ps[:, 0:512],
                    lhsT=w_sl(k, j, which),
                    rhs=xT_sb[k][:, cols], start=(k == 0), stop=False)

        def f2():
            ps = box["ps"]
            for k in range(3, 6):
                nc.tensor.matmul(
                    ps[:, 0:512],
                    lhsT=w_sl(k, j, which),
                    rhs=xT_sb[k][:, cols], start=False, stop=(k == 5))
            nc.vector.tensor_copy(dst, ps[:, 0:512])

        return f1, f2

    for pj in range(NP):
        last_pair = pj == NP - 1
        qt, kt = qT_sb[pj], kT_sb[pj]
        nat3 = natkq[pj].rearrange("p (g t c) -> p g t c", g=2, c=P)
        den = [p_den.tile([P, NT], FP, name=f"den{par}") for par in range(2)]
        rv = [p_den.tile([P, NT], FP, name=f"rv{par}") for par in range(2)]
        Ep = [[None] * NU for _ in range(2)]
        kqs_t = [[None] * NU for _ in range(2)]

        def kqs_u(par, u, nat3=nat3, rv=rv, kqs_t=kqs_t):
            # DVE-only: safe to run inline right after the per-u recip
            dve = (par, u) in DVE_U
            kqs = p_kqs.tile([P, 2, 2 * Z], BF if dve else F8, name="kqst")
            kqs_t[par][u] = kqs
            for i in range(2):
                t = 2 * u + i
                nc.vector.tensor_scalar(
                    kqs[:, i, :].rearrange("p (g z) -> p g z", g=2),
                    nat3[:, :, t, ts(par, Z)],
                    rv[par][:, t:t + 1], float(1 << KQS_SH),
                    op0=mybir.AluOpType.mult,
                    op1=mybir.AluOpType.mult)

        def at_chunk(par, ch, pj=pj, Ep=Ep, kqs_t=kqs_t,
                     last_pair=last_pair):
            # one 512-col half of a head's AT accumulation: 4 DR matmuls
            # (8 bf16 for DVE u-groups) through a 1-bank buffer, then a
            # scaled PSUM->SBUF evacuation of that half.  The last
            # pair's par-1 half uses the chain bank (free by then) so
            # the two heads' tail chains run decoupled.
            def work():
                if last_pair and par == 1:
                    at_ps = psC.tile([P, 512], FP, name="c", tag="c")
                else:
                    at_ps = psA.tile([P, 512], FP, name="at", tag="at")
                cols = slice(512 * ch, 512 * ch + 512)
                for u in range(NU):
                    if (par, u) in DVE_U:
                        for i in range(2):
                            nc.tensor.matmul(
                                at_ps[:], lhsT=kqs_t[par][u][:, i, :],
                                rhs=Ep[par][u][:, i, cols],
                                start=(u == 0 and i == 0),
                                stop=(u == NU - 1 and i == 1),
                            )
                    else:
                        nc.tensor.matmul(
                            at_ps[:], lhsT=kqs_t[par][u][:, 0:2, :],
                            rhs=Ep[par][u][:, 0:2, cols],
                            start=(u == 0), stop=(u == NU - 1),
                            perf_mode=DR,
                        )
                h = 2 * pj + par
                if ch == 0:
                    nc.scalar.mul(AT_sb[h][:, cols], at_ps[:],
                                  2.0 ** (-KQS_SH))
                else:
                    nc.vector.tensor_scalar_mul(AT_sb[h][:, cols], at_ps[:],
                                                2.0 ** (-KQS_SH))
            return work

        # extras: proj chains + nat DMA for pair pj+1
        ext = []
        if not last_pair:
            for which, ch in [(1, 0), (0, 0), (1, 1), (0, 1)]:
                ext.extend(chain_frags(pj + 1, which, ch))
            ext.append(lambda j=pj: emit_nat_dma(j + 1))

        for t in range(NT):
            u, i = t // 2, t & 1
            S2 = []
            for par in range(2):
                S = ps_tile()
                S2.append(S)
                if i == 0:
                    dve = (par, u) in DVE_U
                    Ep[par][u] = (p_Eb.tile([P, 2, N], BF, name="Eb")
                                  if dve else p_E.tile([P, 2, N], F8, name="Ep"))
            # interleaved score MMs: the two heads target PE row groups
            # (0,0) / (64,0) (auto-derived from base_partition) and run
            # concurrently in the array
            for ch in range(2):
                cols = slice(512 * ch, 512 * ch + 512)
                for par in range(2):
                    base = Z * par
                    nc.tensor.matmul(
                        S2[par][:, cols],
                        lhsT=qt[base:base + Z, ts(t, P)],
                        rhs=kt[base:base + Z, cols],
                        start=True, stop=True,
                    )
            for par in range(2):
                E_ap = Ep[par][u][:, i, :]
                if (par, u) in DVE_U:
                    # bf16 Schraudolph bit-trick + DVE row-sum
                    nc.vector.tensor_scalar(
                        E_ap.bitcast(I16), S2[par][:], EXP_K1, EXP_K2,
                        op0=mybir.AluOpType.mult, op1=mybir.AluOpType.add)
                    nc.vector.tensor_reduce(
                        den[par][:, t:t + 1], E_ap,
                        axis=mybir.AxisListType.X, op=mybir.AluOpType.add)
                else:
                    nc.scalar.activation(
                        E_ap, S2[par][:], mybir.ActivationFunctionType.Exp,
                        scale=SCALE, accum_out=den[par][:, t:t + 1],
                    )
            # per-u reciprocals + inline kqs (DVE-only, ready right
            # after the u-group's second exp)
            if i == 1:
                for par in range(2):
                    nc.vector.reciprocal(rv[par][:, 2 * u:2 * u + 2],
                                         den[par][:, 2 * u:2 * u + 2])
                    kqs_u(par, u)
            if last_pair:
                drain_at(1 if t < NT - 1 else 0)
            else:
                drain_at(3 - t // 2)
            for _ in range(2):
                if ext:
                    ext.pop(0)()

        # this pair's AT matmul chunks drain through the next pair
        for par in range(2):
            for ch in range(2):
                at_queue.append(at_chunk(par, ch))
    drain_at(0)

    free_through("natkq0")  # frees xT, wqkT, kT*, qT*, natkq*, warm_sb

    # ---------------- phase C: fused combine + projection + bias ------
    # bias folded in as a K=1 matmul (ones^T @ bp_row) so the PSUM
    # evacuation is a plain copy, alternated between ACT and DVE
    for t in range(NT):
        F_ps = ps_tile()
        for h in range(H):
            for off, w in CCH:
                nc.tensor.matmul(
                    F_ps[:, off:off + w],
                    lhsT=AT_sb[h][:, ts(t, P)],
                    rhs=M_sb[h][:, off:off + w],
                    start=(h == 0), stop=False,
                )
        for off, w in CCH:
            nc.tensor.matmul(
                F_ps[:, off:off + w],
                lhsT=ones_sb[:],
                rhs=bp_sb[0:1, off:off + w],
                start=False, stop=True,
            )
        o = p_out.tile([P, C], FP, name="outt")
        if t % 2 == 0:
            nc.scalar.copy(o[:], F_ps[:, 0:C])
        else:
            nc.vector.tensor_copy(o[:], F_ps[:, 0:C])
        nc.sync.dma_start(out[ts(t, P), :], o[:])

    while stack:
        stack.pop()[1]()


def build():
    nc = bacc.Bacc("TRN2", target_bir_lowering=False, debug=False, num_devices=B)
    xT = nc.dram_tensor("xT", [C, N], BF, kind="ExternalInput").ap()
    wqkT = nc.dram_tensor("wqkT", [C, 2 * C], BF, kind="ExternalInput").ap()
    M = nc.dram_tensor("M", [P, H * C], BF, kind="ExternalInput").ap()
    bpr = nc.dram_tensor("bpr", [1, C], BF, kind="ExternalInput").ap()
    out = nc.dram_tensor("out", [N, C], FP, kind="ExternalOutput").ap()
    with tile.TileContext(nc) as tc, ExitStack() as ctx:
        emit(ctx, tc, (xT, wqkT, M, bpr, out))
    nc.compile()
    return nc


def kernel(x, Wq, Wk, Wp, bp, trace=False, **trace_kwargs):
    global last_results
    x = np.asarray(x, dtype=np.float32)
    Wq = np.asarray(Wq, dtype=np.float32)
    Wk = np.asarray(Wk, dtype=np.float32)
    Wp = np.asarray(Wp, dtype=np.float32)
    bp = np.asarray(bp, dtype=np.float32)

    nc = build()
    bf = ml_dtypes.bfloat16
    wqkTc = np.ascontiguousarray(
        np.concatenate([Wq.T, Wk.T], axis=1)).astype(bf)  # [C, 2C]
    # fused combine+projection weights: M_hT = [Wq_h; Wk_h] @ Wp^T  [2Z, C]
    Wq_h = Wq.reshape(H, Z, C)
    Wk_h = Wk.reshape(H, Z, C)
    W2 = np.concatenate([Wq_h, Wk_h], axis=1)             # [H, 2Z, C]
    M_np = np.einsum("hzc,dc->hzd", W2, Wp)               # [H, 2Z, C]
    Mc = np.ascontiguousarray(
        M_np.transpose(1, 0, 2).reshape(P, H * C)).astype(bf)
    bprc = np.ascontiguousarray(bp.reshape(1, C).astype(bf))
    in_maps = []
    for b in range(B):
        in_maps.append({
            "xT": np.ascontiguousarray(x[b].T).astype(bf),
            "wqkT": wqkTc, "M": Mc, "bpr": bprc,
        })
    res = bass_utils.run_bass_kernel_spmd(
        nc, in_maps, core_ids=list(range(B)), trace=trace, **trace_kwargs)
    last_results = res
    return np.stack([res.results[b]["out"] for b in range(B)], axis=0)
